# revision 1
# baseline (speedup 1.0000x reference)
"""BatchedLightSimulation Trainium2 kernel.

Math: the two causal convolutions (scintillation 990 taps, SiPM impulse 990
taps) compose into one 1979-tap causal filter c.  Folding the sum-by-16
downsample in gives

    out[row, s] = gain[row] * sum_delta c16[delta] * u[row, 16*s + delta]

with c16[delta] = sum_{k=max(0,delta)}^{15} c[k - delta].  c decays like
exp(-l/15.3) so c16 truncated to delta >= -240 is exact at fp32 precision
(validated 4e-7 of absmax vs the jax reference).

Device mapping (per core, 4 ninputs = 192 (n,d) rows):
  polyphase m = 16q + r.  SBUF tile X[q, row, r] holds 64B chunks
  u[row, 16q:16q+16].  For each output tile of 100 s-values and each phase
  r, one fp32 matmul accumulates W_r[q_rel, s_rel].T @ X[:, :, r] into
  psum[100, 192]; 16 phases x 4 s-tiles = 64 matmuls.  q-window per s-tile
  is [s0-15, s0+99] (115 partitions); the host pads the time axis by 240
  zeros so the window never underflows.  Epilogue: gain multiply (DVE),
  PE transpose to [row, s], DMA out.
"""

import numpy as np

import concourse.bacc as bacc
import concourse.mybir as mybir
import concourse.tile as tile
from concourse.bass_utils import run_bass_kernel_spmd

# ---- problem constants (hardcoded per contract) ----
NINPUT, NDET, NTICK = 32, 48, 6400
NS = 16                    # downsample factor
S = NTICK // NS            # 400 output ticks
LIGHT_TICK = 0.1
CONV_TICKS = 990
NCORES = 8
N_PER_CORE = NINPUT // NCORES      # 4
ROWS = N_PER_CORE * NDET           # 192 rows per core
J = 15                             # q-steps of history (taps delta >= -16*J)
HALO = J
PAD = NS * HALO                    # 240 zero ticks prepended
TPAD = NTICK + PAD                 # 6640
STILE = 100                        # s-values per output tile
NST = S // STILE                   # 4
QW = STILE + HALO                  # 115 q-partitions per tile
DMAX = NS * J                      # 240


def _build_taps(singlet_fraction_logit, log_tau_s, log_tau_t,
                light_oscillation_period, light_response_time):
    """c16[delta] for delta in [-DMAX, 15], float64."""
    dt = float(LIGHT_TICK)
    tt = np.arange(CONV_TICKS, dtype=np.float64)
    sf = 1.0 / (1.0 + np.exp(-float(singlet_fraction_logit)))
    tau_s = 10.0 ** float(log_tau_s)
    tau_t = 10.0 ** float(log_tau_t)
    per = float(light_oscillation_period)
    rt = float(light_response_time)
    p1 = sf * np.exp(-tt * dt / tau_s) * (1.0 - np.exp(-dt / tau_s))
    p3 = (1.0 - sf) * np.exp(-tt * dt / tau_t) * (1.0 - np.exp(-dt / tau_t))
    scint = p1 + p3
    t = tt * dt
    imp = np.exp(-t / rt) * np.sin(t / per)
    imp = imp / (per * rt * rt) * (per * per + rt * rt) * dt
    c = np.convolve(scint, imp)          # length 2*990-1 = 1979
    deltas = np.arange(-DMAX, 16)
    c16 = np.zeros(len(deltas), dtype=np.float64)
    for i, d in enumerate(deltas):
        ks = np.arange(max(0, d), 16)
        c16[i] = c[ks - d].sum()
    return c16                            # index i -> delta = i - DMAX


def _build_weights(c16):
    """W[q_rel, r, s_rel] float32, shared by all four s-tiles."""
    w = np.zeros((QW, NS, STILE), dtype=np.float64)
    q_rel = np.arange(QW)[:, None, None]
    r = np.arange(NS)[None, :, None]
    s_rel = np.arange(STILE)[None, None, :]
    delta = 16 * (q_rel - HALO - s_rel) + r
    mask = (delta >= -DMAX) & (delta <= 15)
    w[mask] = c16[(delta + DMAX)[mask]]
    return np.ascontiguousarray(w, dtype=np.float32)


_PROGRAM = None


def _build_program():
    global _PROGRAM
    if _PROGRAM is not None:
        return _PROGRAM
    nc = bacc.Bacc("TRN2", target_bir_lowering=False, debug=False,
                   num_devices=NCORES)
    f32 = mybir.dt.float32
    u_d = nc.dram_tensor("u", [ROWS, TPAD], f32, kind="ExternalInput")
    w_d = nc.dram_tensor("w", [QW, NS * STILE], f32, kind="ExternalInput")
    g_d = nc.dram_tensor("gain", [128, ROWS], f32, kind="ExternalInput")
    i_d = nc.dram_tensor("ident", [128, 128], f32, kind="ExternalInput")
    o_d = nc.dram_tensor("out", [ROWS, S], f32, kind="ExternalOutput")

    with tile.TileContext(nc) as tc:
        with (
            tc.tile_pool(name="const", bufs=1) as cpool,
            tc.tile_pool(name="x", bufs=1) as xpool,
            tc.tile_pool(name="ep", bufs=2) as epool,
            tc.tile_pool(name="fin", bufs=1) as fpool,
            tc.tile_pool(name="ps", bufs=2, space="PSUM") as pspool,
            tc.tile_pool(name="psT", bufs=2, space="PSUM") as ptpool,
        ):
            w_sb = cpool.tile([QW, NS * STILE], f32, tag="w")
            nc.sync.dma_start(w_sb[:], w_d[:])
            g_sb = cpool.tile([128, ROWS], f32, tag="g")
            nc.sync.dma_start(g_sb[:], g_d[:])
            id_sb = cpool.tile([128, 128], f32, tag="id")
            nc.sync.dma_start(id_sb[:], i_d[:])

            x_sb = []
            for st in range(NST):
                x = xpool.tile([QW, ROWS, NS], f32, tag=f"x{st}")
                src = u_d[:, NS * STILE * st: NS * STILE * st + NS * QW]
                src = src.rearrange("row (q r) -> q row r", r=NS)
                nc.sync.dma_start(x[:], src)
                x_sb.append(x)

            fin_a = fpool.tile([128, S], f32, tag="fa")
            fin_b = fpool.tile([64, S], f32, tag="fb")

            for st in range(NST):
                ps = pspool.tile([STILE, ROWS], f32, tag="ps")
                for r in range(NS):
                    nc.tensor.matmul(
                        ps[:],
                        w_sb[:, r * STILE:(r + 1) * STILE],
                        x_sb[st][:, :, r],
                        start=(r == 0),
                        stop=(r == NS - 1),
                    )
                gained = epool.tile([STILE, ROWS], f32, tag="gained")
                nc.vector.tensor_mul(gained[:], ps[:], g_sb[0:STILE, :])
                pT_a = ptpool.tile([128, STILE], f32, tag="pTa")
                nc.tensor.transpose(pT_a[:], gained[:, 0:128],
                                    id_sb[0:STILE, 0:STILE])
                pT_b = ptpool.tile([64, STILE], f32, tag="pTb")
                nc.tensor.transpose(pT_b[:], gained[:, 128:ROWS],
                                    id_sb[0:STILE, 0:STILE])
                nc.vector.tensor_copy(fin_a[:, st * STILE:(st + 1) * STILE],
                                      pT_a[:])
                nc.vector.tensor_copy(fin_b[:, st * STILE:(st + 1) * STILE],
                                      pT_b[:])

            nc.sync.dma_start(o_d[0:128, :], fin_a[:])
            nc.sync.dma_start(o_d[128:ROWS, :], fin_b[:])

    nc.compile()
    _PROGRAM = nc
    return nc


def _prepare_inputs(timing_dist, singlet_fraction_logit, log_tau_s, log_tau_t,
                    light_oscillation_period, light_response_time, light_gain):
    u = np.ascontiguousarray(np.asarray(timing_dist, dtype=np.float32))
    assert u.shape == (NINPUT, NDET, NTICK)
    gain = np.asarray(light_gain, dtype=np.float32).reshape(NDET)

    c16 = _build_taps(singlet_fraction_logit, log_tau_s, log_tau_t,
                      light_oscillation_period, light_response_time)
    w = _build_weights(c16).reshape(QW, NS * STILE)

    gain_row = np.tile(gain, N_PER_CORE)                     # [ROWS]
    gain_rep = np.ascontiguousarray(
        np.broadcast_to(gain_row[None, :], (128, ROWS)), dtype=np.float32)
    ident = np.eye(128, dtype=np.float32)

    in_maps = []
    for c in range(NCORES):
        shard = u[c * N_PER_CORE:(c + 1) * N_PER_CORE].reshape(ROWS, NTICK)
        up = np.zeros((ROWS, TPAD), dtype=np.float32)
        up[:, PAD:] = shard
        in_maps.append({"u": up, "w": w, "gain": gain_rep, "ident": ident})
    return in_maps


def _run(in_maps, trace=False):
    nc = _build_program()
    res = run_bass_kernel_spmd(nc, in_maps, core_ids=list(range(NCORES)),
                               trace=trace)
    outs = [res.results[c]["out"].reshape(N_PER_CORE, NDET, S)
            for c in range(NCORES)]
    full = np.concatenate(outs, axis=0).astype(np.float32, copy=False)
    return full, res


def kernel(timing_dist, singlet_fraction_logit, log_tau_s, log_tau_t,
           light_oscillation_period, light_response_time, light_gain):
    in_maps = _prepare_inputs(
        timing_dist, singlet_fraction_logit, log_tau_s, log_tau_t,
        light_oscillation_period, light_response_time, light_gain)
    full, _ = _run(in_maps, trace=False)
    return full


# revision 2
# speedup vs baseline: 2.3686x; 2.3686x over previous
"""BatchedLightSimulation Trainium2 kernel.

Math: the two causal convolutions (scintillation 990 taps, SiPM impulse 990
taps) compose into one 1979-tap causal filter c.  Folding the sum-by-16
downsample in gives

    out[row, s] = gain[row] * sum_delta c16[delta] * u[row, 16*s + delta]

with c16[delta] = sum_{k=max(0,delta)}^{15} c[k - delta].  c decays like
exp(-l/15.3) so c16 truncated to delta >= -240 is exact at fp32 precision
(validated 4e-7 of absmax vs the jax reference).

Device mapping (per core, 4 ninputs = 192 (n,d) rows):
  polyphase m = 16q + r.  SBUF tile X[q, row, r] holds the 64B chunks
  u[row, 16q:16q+16].  For each output tile of 100 s-values and each phase
  r, one fp32 matmul accumulates W_r[q_rel, s_rel].T @ X[:, :, r] into
  psum[100, rows]; 16 phases x 4 s-tiles = 64 matmuls.  The q-window per
  s-tile is [s0-15, s0+99] (115 partitions); the time axis is padded by
  240 zeros so the window never underflows.  Epilogue: gain multiply
  (DVE), PE transpose to [row, s], DMA out.

The host ships each core's shard already in the [s-tile, q, row, r]
polyphase layout (a pure permutation done during the shard-and-copy step)
so the input DMA is fully contiguous; a 64B-chunk gather DMA measures
~38 GB/s on TRN2 vs ~300 GB/s contiguous.
"""

import numpy as np

import concourse.bacc as bacc
import concourse.mybir as mybir
import concourse.tile as tile
from concourse.bass_utils import run_bass_kernel_spmd

# ---- problem constants (hardcoded per contract) ----
NINPUT, NDET, NTICK = 32, 48, 6400
NS = 16                    # downsample factor
S = NTICK // NS            # 400 output ticks
LIGHT_TICK = 0.1
CONV_TICKS = 990
NCORES = 8
N_PER_CORE = NINPUT // NCORES      # 4
ROWS = N_PER_CORE * NDET           # 192 rows per core
J = 15                             # q-steps of history (taps delta >= -16*J)
HALO = J
PAD = NS * HALO                    # 240 zero ticks prepended
STILE = 100                        # s-values per output tile
NST = S // STILE                   # 4
QW = STILE + HALO                  # 115 q-partitions per tile
DMAX = NS * J                      # 240
RPAD = 256                         # rhs free-dim padding (fp32r full rate)

USE_FP32R = False


def _build_taps(singlet_fraction_logit, log_tau_s, log_tau_t,
                light_oscillation_period, light_response_time):
    """c16[delta] for delta in [-DMAX, 15], float64."""
    dt = float(LIGHT_TICK)
    tt = np.arange(CONV_TICKS, dtype=np.float64)
    sf = 1.0 / (1.0 + np.exp(-float(singlet_fraction_logit)))
    tau_s = 10.0 ** float(log_tau_s)
    tau_t = 10.0 ** float(log_tau_t)
    per = float(light_oscillation_period)
    rt = float(light_response_time)
    p1 = sf * np.exp(-tt * dt / tau_s) * (1.0 - np.exp(-dt / tau_s))
    p3 = (1.0 - sf) * np.exp(-tt * dt / tau_t) * (1.0 - np.exp(-dt / tau_t))
    scint = p1 + p3
    t = tt * dt
    imp = np.exp(-t / rt) * np.sin(t / per)
    imp = imp / (per * rt * rt) * (per * per + rt * rt) * dt
    c = np.convolve(scint, imp)          # length 2*990-1 = 1979
    deltas = np.arange(-DMAX, 16)
    c16 = np.zeros(len(deltas), dtype=np.float64)
    for i, d in enumerate(deltas):
        ks = np.arange(max(0, d), 16)
        c16[i] = c[ks - d].sum()
    return c16                            # index i -> delta = i - DMAX


def _build_weights(c16):
    """W[q_rel, r, s_rel] float32, shared by all four s-tiles."""
    w = np.zeros((QW, NS, STILE), dtype=np.float64)
    q_rel = np.arange(QW)[:, None, None]
    r = np.arange(NS)[None, :, None]
    s_rel = np.arange(STILE)[None, None, :]
    delta = 16 * (q_rel - HALO - s_rel) + r
    mask = (delta >= -DMAX) & (delta <= 15)
    w[mask] = c16[(delta + DMAX)[mask]]
    return np.ascontiguousarray(w, dtype=np.float32)


_PROGRAM = None


def _build_program():
    global _PROGRAM
    if _PROGRAM is not None:
        return _PROGRAM
    nc = bacc.Bacc("TRN2", target_bir_lowering=False, debug=False,
                   num_devices=NCORES)
    f32 = mybir.dt.float32
    rhs_dt = mybir.dt.float32r if USE_FP32R else mybir.dt.float32
    x_d = nc.dram_tensor("x", [NST, QW, ROWS * NS], f32, kind="ExternalInput")
    w_d = nc.dram_tensor("w", [QW, NS * STILE], f32, kind="ExternalInput")
    g_d = nc.dram_tensor("gain", [128, ROWS], f32, kind="ExternalInput")
    i_d = nc.dram_tensor("ident", [128, 128], f32, kind="ExternalInput")
    o_d = nc.dram_tensor("out", [ROWS, S], f32, kind="ExternalOutput")

    with tile.TileContext(nc) as tc:
        with (
            tc.tile_pool(name="const", bufs=1) as cpool,
            tc.tile_pool(name="x", bufs=1) as xpool,
            tc.tile_pool(name="ep", bufs=2) as epool,
            tc.tile_pool(name="fin", bufs=1) as fpool,
            tc.tile_pool(name="ps", bufs=2, space="PSUM") as pspool,
            tc.tile_pool(name="psT", bufs=2, space="PSUM") as ptpool,
        ):
            w_sb = cpool.tile([QW, NS * STILE], rhs_dt, tag="w")
            nc.sync.dma_start(w_sb[:], w_d[:])
            g_sb = cpool.tile([128, ROWS], f32, tag="g")
            nc.sync.dma_start(g_sb[:], g_d[:])
            id_sb = cpool.tile([128, 128], f32, tag="id")
            nc.sync.dma_start(id_sb[:], i_d[:])

            x_sb = []
            for st in range(NST):
                if USE_FP32R:
                    x = xpool.tile([QW, RPAD, NS], rhs_dt, tag=f"x{st}")
                    nc.sync.dma_start(
                        x[:, 0:ROWS, :].rearrange("q row r -> q (row r)"),
                        x_d[st])
                else:
                    x = xpool.tile([QW, ROWS, NS], rhs_dt, tag=f"x{st}")
                    nc.sync.dma_start(
                        x[:].rearrange("q row r -> q (row r)"), x_d[st])
                x_sb.append(x)

            fin_a = fpool.tile([128, S], f32, tag="fa")
            fin_b = fpool.tile([64, S], f32, tag="fb")

            nrhs = RPAD if USE_FP32R else ROWS
            for st in range(NST):
                ps = pspool.tile([STILE, nrhs], f32, tag="ps")
                for r in range(NS):
                    nc.tensor.matmul(
                        ps[:],
                        w_sb[:, r * STILE:(r + 1) * STILE],
                        x_sb[st][:, :, r],
                        start=(r == 0),
                        stop=(r == NS - 1),
                    )
                gained = epool.tile([STILE, ROWS], f32, tag="gained")
                nc.vector.tensor_mul(gained[:], ps[:, 0:ROWS],
                                     g_sb[0:STILE, :])
                pT_a = ptpool.tile([128, STILE], f32, tag="pTa")
                nc.tensor.transpose(pT_a[:], gained[:, 0:128],
                                    id_sb[0:STILE, 0:STILE])
                pT_b = ptpool.tile([64, STILE], f32, tag="pTb")
                nc.tensor.transpose(pT_b[:], gained[:, 128:ROWS],
                                    id_sb[0:STILE, 0:STILE])
                nc.vector.tensor_copy(fin_a[:, st * STILE:(st + 1) * STILE],
                                      pT_a[:])
                nc.vector.tensor_copy(fin_b[:, st * STILE:(st + 1) * STILE],
                                      pT_b[:])

            nc.sync.dma_start(o_d[0:128, :], fin_a[:])
            nc.sync.dma_start(o_d[128:ROWS, :], fin_b[:])

    nc.compile()
    _PROGRAM = nc
    return nc


def _prepare_inputs(timing_dist, singlet_fraction_logit, log_tau_s, log_tau_t,
                    light_oscillation_period, light_response_time, light_gain):
    u = np.ascontiguousarray(np.asarray(timing_dist, dtype=np.float32))
    assert u.shape == (NINPUT, NDET, NTICK)
    gain = np.asarray(light_gain, dtype=np.float32).reshape(NDET)

    c16 = _build_taps(singlet_fraction_logit, log_tau_s, log_tau_t,
                      light_oscillation_period, light_response_time)
    w = _build_weights(c16).reshape(QW, NS * STILE)

    gain_row = np.tile(gain, N_PER_CORE)                     # [ROWS]
    gain_rep = np.ascontiguousarray(
        np.broadcast_to(gain_row[None, :], (128, ROWS)), dtype=np.float32)
    ident = np.eye(128, dtype=np.float32)

    in_maps = []
    for c in range(NCORES):
        shard = u[c * N_PER_CORE:(c + 1) * N_PER_CORE].reshape(ROWS, NTICK)
        up = np.zeros((ROWS, TPAD), dtype=np.float32)
        up[:, PAD:] = shard
        # polyphase relayout: x[st, q, row, r] = up[row, 16*(st*100) + 16*q + r]
        xv = np.lib.stride_tricks.as_strided(
            up,
            shape=(NST, QW, ROWS, NS),
            strides=(NS * STILE * 4, NS * 4, up.strides[0], 4),
        )
        x = np.ascontiguousarray(xv, dtype=np.float32).reshape(
            NST, QW, ROWS * NS)
        in_maps.append({"x": x, "w": w, "gain": gain_rep, "ident": ident})
    return in_maps


TPAD = NTICK + PAD                 # 6640


def _run(in_maps, trace=False):
    nc = _build_program()
    res = run_bass_kernel_spmd(nc, in_maps, core_ids=list(range(NCORES)),
                               trace=trace)
    outs = [res.results[c]["out"].reshape(N_PER_CORE, NDET, S)
            for c in range(NCORES)]
    full = np.concatenate(outs, axis=0).astype(np.float32, copy=False)
    return full, res


def kernel(timing_dist, singlet_fraction_logit, log_tau_s, log_tau_t,
           light_oscillation_period, light_response_time, light_gain):
    in_maps = _prepare_inputs(
        timing_dist, singlet_fraction_logit, log_tau_s, log_tau_t,
        light_oscillation_period, light_response_time, light_gain)
    full, _ = _run(in_maps, trace=False)
    return full
